# revision 1
# baseline (speedup 1.0000x reference)
"""Trainium2 Bass kernel for nn_FullAttentionBlock (B=4, N=1024, D=1024, H=16).

Sharding: 8 cores; core c handles batch c//2, query-row half c%2 (512 rows).
Each core: LN1 + QKV for its whole batch (K/V need all rows), full attention
for all 16 heads over its 512 query rows, out-proj + residual, LN2 + SwiGLU
MLP for its rows. No collectives.

Host prep (weights only + rope/spatial tables):
 - LN/RMS weights and LayerScale gains folded into matmul weights.
 - RoPE pair permutation: within each head, dim order (0,2,..62,1,3,..63) so
   the rotation halves are contiguous; q.k dot products are invariant.
 - Spatial bias collapses to rank-3: bias_h = coords @ A_h @ coords^T with
   A_h = sp_scale * sq_w_h^T @ sk_w_h (3x3).  q/k are augmented with 3 extra
   channels (padded to 128) so scores are one 128-contraction matmul per tile.
 - Softmax denominator via an extra all-ones column in V.
"""

import os
from contextlib import ExitStack

import numpy as np
import ml_dtypes

import concourse.bass as bass
import concourse.tile as tile
from concourse import bacc, mybir
from concourse.bass_utils import run_bass_kernel_spmd

B, N, D, H = 4, 1024, 1024, 16
HD = 64
HID = 4 * D
EPS = 1e-6
THETA = 10000.0
SP_SCALE = 1.0
P = 128
NCORES = 8
MY = N // 2  # 512 query rows per core

bf16 = ml_dtypes.bfloat16
BF = mybir.dt.bfloat16
F32 = mybir.dt.float32
AF = mybir.ActivationFunctionType
OP = mybir.AluOpType


def _f32(x):
    return np.ascontiguousarray(np.asarray(x, np.float32))


def _bf(x):
    return np.ascontiguousarray(np.asarray(x, np.float32).astype(bf16))


# ---------------------------------------------------------------------------
# device program
# ---------------------------------------------------------------------------

def build_program(with_b1=False, with_bo1=False, with_bo2=False):
    nc = bacc.Bacc(
        "TRN2",
        target_bir_lowering=False,
        debug=False,
        enable_asserts=False,
        num_devices=NCORES,
    )

    # --- dram I/O ---
    x_d = nc.dram_tensor("x", (N, D), F32, kind="ExternalInput").ap()
    xbf_d = nc.dram_tensor("x_bf", (N, D), BF, kind="ExternalInput").ap()
    csum_d = nc.dram_tensor("csum_rep", (P, 3 * D), BF, kind="ExternalInput").ap()
    coords_d = nc.dram_tensor("coords_tm", (N, 3), BF, kind="ExternalInput").ap()
    coordsT_d = nc.dram_tensor("coordsT", (3, N), BF, kind="ExternalInput").ap()
    acat_d = nc.dram_tensor("a_cat", (3, H * 3), BF, kind="ExternalInput").ap()
    tq_d = nc.dram_tensor("tq", (MY, 2, 64), BF, kind="ExternalInput").ap()
    tk_d = nc.dram_tensor("tk", (N, 2, 64), BF, kind="ExternalInput").ap()
    wqT_d = nc.dram_tensor("wqT", (D, D), BF, kind="ExternalInput").ap()
    wkT_d = nc.dram_tensor("wkT", (D, D), BF, kind="ExternalInput").ap()
    wvT_d = nc.dram_tensor("wvT", (D, D), BF, kind="ExternalInput").ap()
    owT_d = nc.dram_tensor("owT", (D, D), BF, kind="ExternalInput").ap()
    w2T_d = nc.dram_tensor("w2T", (D, 32, 256), BF, kind="ExternalInput").ap()
    b2_d = nc.dram_tensor("b2", (P, 2 * HID // P), F32, kind="ExternalInput").ap()
    woT_d = nc.dram_tensor("woT", (HID, D), BF, kind="ExternalInput").ap()
    out_d = nc.dram_tensor("out", (MY, D), F32, kind="ExternalOutput").ap()
    if with_b1:
        b1r_d = nc.dram_tensor("b1rep", (P, 3 * D), F32, kind="ExternalInput").ap()
    if with_bo1:
        bo1r_d = nc.dram_tensor("bo1rep", (P, D), F32, kind="ExternalInput").ap()
    if with_bo2:
        bo2r_d = nc.dram_tensor("bo2rep", (P, D), F32, kind="ExternalInput").ap()

    # --- dram scratch ---
    qaug_d = nc.dram_tensor("qaug_s", (MY, H * P), BF, kind="Internal").ap()
    kaug_d = nc.dram_tensor("kaug_s", (N, H * P), BF, kind="Internal").ap()
    xh2_d = nc.dram_tensor("xh2_s", (MY, D), BF, kind="Internal").ap()

    markers = {}
    build_program.markers = markers

    def mark(tag):
        markers[tag] = nc.next_id()

    with tile.TileContext(nc) as tc, ExitStack() as ctx:
        # ---- persistent pools (whole kernel) ----
        pers = ctx.enter_context(tc.tile_pool(name="pers", bufs=1))
        x_res = pers.tile([P, 4, D], F32)        # my x rows (residual)
        vtil = pers.tile([P, 8, H, 65], BF)      # v (token-major) + ones col
        oT_all = pers.tile([P, 8, MY], BF)       # attention out^T, head-major
        x1_sb = pers.tile([P, 4, D], F32)        # x + ls1*o

        small = ctx.enter_context(tc.tile_pool(name="small", bufs=1))
        coordsT_sb = small.tile([3, N], BF)
        acat_sb = small.tile([3, H * 3], BF)
        coords_sb = small.tile([P, 8, 3], BF)
        cq_sb = small.tile([P, 4, H * 3], BF)
        b2_sb = small.tile([P, 2 * HID // P], F32)
        eps_sb = small.tile([P, 1], F32)
        nc.vector.memset(eps_sb[:], EPS)
        nc.sync.dma_start(coordsT_sb[:], coordsT_d)
        nc.sync.dma_start(acat_sb[:], acat_d)
        nc.sync.dma_start(coords_sb[:], coords_d.rearrange("(o p) c -> p o c", p=P))
        nc.sync.dma_start(b2_sb[:], b2_d)
        if with_b1:
            b1r_sb = small.tile([P, 3 * D], F32)
            nc.sync.dma_start(b1r_sb[:], b1r_d)
        if with_bo1:
            bo1r_sb = small.tile([P, D], F32)
            nc.sync.dma_start(bo1r_sb[:], bo1r_d)
        if with_bo2:
            bo2r_sb = small.tile([P, D], F32)
            nc.sync.dma_start(bo2r_sb[:], bo2r_d)

        ln_pool = ctx.enter_context(tc.tile_pool(name="ln", bufs=2))
        st_pool = ctx.enter_context(tc.tile_pool(name="stats", bufs=4))

        def layernorm_chunk(x_ap, out_ap, inv_ap=None, nmi_ap=None):
            """out (bf16) = (x - mean)/sqrt(var + eps); x_ap [128, D] f32."""
            ssum = st_pool.tile([P, 1], F32, tag="ssum")
            nmean = st_pool.tile([P, 1], F32, tag="nmean")
            sq = ln_pool.tile([P, D], BF, tag="lnsq")
            ss = st_pool.tile([P, 1], F32, tag="ss")
            var = st_pool.tile([P, 1], F32, tag="var")
            mu2 = st_pool.tile([P, 1], F32, tag="mu2")
            sd = st_pool.tile([P, 1], F32, tag="sd")
            if inv_ap is None:
                inv_t = st_pool.tile([P, 1], F32, tag="inv")
                inv_ap = inv_t[:]
            if nmi_ap is None:
                nmi_t = st_pool.tile([P, 1], F32, tag="nmi")
                nmi_ap = nmi_t[:]
            inv = inv_ap
            nmi = nmi_ap
            nc.vector.reduce_sum(ssum[:], x_ap, axis=mybir.AxisListType.X)
            nc.vector.tensor_scalar_mul(nmean[:], ssum[:], -1.0 / D)
            nc.scalar.activation(sq[:], x_ap, AF.Square, accum_out=ss[:])
            nc.vector.tensor_tensor(mu2[:], nmean[:], nmean[:], OP.mult)
            nc.vector.scalar_tensor_tensor(
                var[:], ss[:], 1.0 / D, mu2[:], op0=OP.mult, op1=OP.subtract
            )
            nc.scalar.activation(sd[:], var[:], AF.Sqrt, bias=eps_sb[:])
            nc.vector.reciprocal(inv, sd[:])
            nc.vector.tensor_tensor(nmi, nmean[:], inv, OP.mult)
            if out_ap is None:
                return inv, nmi
            nc.scalar.activation(out_ap, x_ap, AF.Identity, bias=nmi, scale=inv)

        # ====== phases A+B+C: LN1-folded qkv, q/k processing ================
        # qkv = inv[t] * (x_bf @ W'^T) - mu[t]*inv[t] * colsum[j]  (LN1 folded
        # into a per-token scalar correction, so the matmul consumes raw x^T
        # transposed straight from the DRAM input with no LN dependency).
        with tc.tile_pool(name="phc", bufs=1) as phc, \
             tc.tile_pool(name="xa", bufs=1) as xa_pool, \
             tc.tile_pool(name="qkproc", bufs=2) as qk_pool, \
             tc.tile_pool(name="cqpsum", bufs=2, space="PSUM") as cq_psum, \
             tc.tile_pool(name="qkvpsum", bufs=3, space="PSUM") as psum:
            xT = phc.tile([P, 8, N], BF)             # raw x^T (bf16)
            wq_sb = phc.tile([P, 8, D], BF)
            wk_sb = phc.tile([P, 8, D], BF)
            wv_sb = phc.tile([P, 8, D], BF)
            wk_r = wkT_d.rearrange("(o p) f -> p o f", p=P)
            # one copy then the transposes: alternating copy/transpose pays
            # the xbar-mode serialization on every switch
            nc.sync.dma_start(wk_sb[:], wk_r)
            nc.sync.dma_start_transpose(xT[:, :, 0:P], xbf_d[0:P, :])
            nc.sync.dma_start_transpose(xT[:, :, P:2 * P], xbf_d[P:2 * P, :])
            csum_sb = phc.tile([P, 3 * D], BF)
            xtm = xa_pool.tile([P, 8, D], BF, name="xtm")
            xtm_r = xbf_d.rearrange("(o p) f -> p o f", p=P)
            nc.sync.dma_start(xtm[:, 0:2], xtm_r[:, 0:2])
            nc.sync.dma_start(csum_sb[:, D:2 * D], csum_d[:, D:2 * D])
            tqtk = phc.tile([P, 12, 2, 64], BF)      # rope tables: q 0..3, k 4..11
            nc.sync.dma_start(tqtk[:, 4:12], tk_d.rearrange("(o p) a b -> p o a b", p=P))
            wq_r = wqT_d.rearrange("(o p) f -> p o f", p=P)
            nc.sync.dma_start(wq_sb[:, 0:4], wq_r[:, 0:4])
            nc.sync.dma_start(xtm[:, 2:4], xtm_r[:, 2:4])
            nc.sync.dma_start(wq_sb[:, 4:8], wq_r[:, 4:8])
            for t8 in range(2, 8):
                nc.sync.dma_start_transpose(
                    xT[:, :, t8 * P:(t8 + 1) * P],
                    xbf_d[t8 * P:(t8 + 1) * P, :],
                )
            nc.sync.dma_start(csum_sb[:, 0:D], csum_d[:, 0:D])
            nc.sync.dma_start(tqtk[:, 0:4], tq_d.rearrange("(o p) a b -> p o a b", p=P))
            nc.sync.dma_start(wv_sb[:], wvT_d.rearrange("(o p) f -> p o f", p=P))
            nc.sync.dma_start(csum_sb[:, 2 * D:], csum_d[:, 2 * D:])
            # ping-pong aug buffers: pad cols 67..128 stay zero across reuse
            aug_bufs = [phc.tile([P, H, P], BF, name=f"augbuf{i}")
                        for i in range(2)]
            nc.vector.memset(aug_bufs[0][:], 0.0)
            nc.vector.memset(aug_bufs[1][:], 0.0)
            nc.vector.memset(vtil[:, :, :, 64:65], 1.0)

            nc.sync.dma_start(xtm[:, 4:8], xtm_r[:, 4:8])
            inv8 = phc.tile([P, 8], F32)
            nmi8 = phc.tile([P, 8], F32)
            # batched LN1 stats, two groups of 4 chunks
            ssum8 = phc.tile([P, 8], F32)
            ss8 = phc.tile([P, 8], F32)
            sq_s = phc.tile([P, D], BF)
            for grp in range(4):
                for i in range(2):
                    t8 = grp * 2 + i
                    x_ap = xtm[:, t8, :]
                    nc.vector.reduce_sum(ssum8[:, t8:t8 + 1], x_ap,
                                         axis=mybir.AxisListType.X)
                    nc.scalar.activation(sq_s[:], x_ap, AF.Square,
                                         accum_out=ss8[:, t8:t8 + 1])
                g = slice(grp * 2, grp * 2 + 2)
                nmean = st_pool.tile([P, 2], F32, tag="b_nmean")
                mu2 = st_pool.tile([P, 2], F32, tag="b_mu2")
                var = st_pool.tile([P, 2], F32, tag="b_var")
                sd = st_pool.tile([P, 2], F32, tag="b_sd")
                nc.vector.tensor_scalar_mul(nmean[:], ssum8[:, g], -1.0 / D)
                nc.vector.tensor_tensor(mu2[:], nmean[:], nmean[:], OP.mult)
                nc.vector.scalar_tensor_tensor(
                    var[:], ss8[:, g], 1.0 / D, mu2[:],
                    op0=OP.mult, op1=OP.subtract)
                nc.scalar.activation(sd[:], var[:], AF.Sqrt, bias=eps_sb[:])
                nc.vector.reciprocal(inv8[:, g], sd[:])
                nc.vector.tensor_tensor(nmi8[:, g], nmean[:], inv8[:, g],
                                        OP.mult)
            stats = [(inv8[:, t:t + 1], nmi8[:, t:t + 1]) for t in range(8)]

            # cq = coords @ A_cat  (my 4 chunks) -> sbuf
            for t4 in range(4):
                cp = cq_psum.tile([P, H * 3], F32, tag="cqp")
                nc.tensor.matmul(
                    cp[:], coordsT_sb[:, t4 * P:(t4 + 1) * P], acat_sb[:],
                    start=True, stop=True,
                )
                nc.scalar.copy(cq_sb[:, t4, :], cp[:])

            def proj_corrected(w_sb, t8, which, bias_off=None, out_ap=None):
                """ln-corrected token-major projection [128, D] bf16 sbuf."""
                ps = psum.tile([P, D], F32, tag="qkv_ps")
                for hf in range(2):
                    for dc in range(8):
                        nc.tensor.matmul(
                            ps[:, hf * 512:(hf + 1) * 512],
                            xT[:, dc, t8 * P:(t8 + 1) * P],
                            w_sb[:, dc, hf * 512:(hf + 1) * 512],
                            start=(dc == 0),
                            stop=(dc == 7),
                        )
                inv, nmi = stats[t8]
                ta = qk_pool.tile([P, D], BF, tag="ta")
                nc.scalar.activation(ta[:], ps[:], AF.Identity, scale=inv)
                co = which * D
                if out_ap is None:
                    dst = qk_pool.tile([P, D], BF, tag="corr")
                    out_ap = dst[:]
                    csrc = csum_sb[:, co:co + D]
                    tsrc = ta[:]
                else:
                    csrc = csum_sb[:, co:co + D].rearrange(
                        "p (h d) -> p h d", h=H)
                    tsrc = ta[:].rearrange("p (h d) -> p h d", h=H)
                nc.vector.scalar_tensor_tensor(
                    out_ap, csrc, nmi, tsrc, op0=OP.mult, op1=OP.add,
                )
                if bias_off is not None:
                    nc.vector.tensor_tensor(
                        out_ap, out_ap, b1r_sb[:, bias_off:bias_off + D], OP.add
                    )
                return out_ap

            def qk_process(src_ap, tbl_i, t8, aug_fn, dst_dram):
                """rms-norm + rope on token-major q/k chunk; writes aug tile."""
                sqs = qk_pool.tile([P, D], BF, tag="sqs")
                nc.scalar.activation(sqs[:], src_ap, AF.Square)
                ss = st_pool.tile([P, H], F32, tag="rms_ss")
                nc.vector.reduce_sum(
                    ss[:], sqs[:].rearrange("p (h d) -> p h d", h=H),
                    axis=mybir.AxisListType.X,
                )
                sd = st_pool.tile([P, H], F32, tag="rms_sd")
                nc.scalar.activation(sd[:], ss[:], AF.Sqrt, scale=1.0 / HD,
                                     bias=eps_sb[:])
                rs = st_pool.tile([P, H], F32, tag="rms_rs")
                nc.vector.reciprocal(rs[:], sd[:])
                qs = qk_pool.tile([P, H, HD], BF, tag="qs")
                nc.vector.tensor_tensor(
                    qs[:], src_ap.rearrange("p (h d) -> p h d", h=H),
                    rs[:, :, None].to_broadcast((P, H, HD)), OP.mult,
                )
                aug = aug_bufs[qk_process.flip][:]
                qk_process.flip ^= 1
                # rope via 2 ops: prods[p,h,j,d] = qs[p,h,d] * tb[p,j,d]
                # (tb[0] = [cos*w1 | -sin*w2], tb[1] = [sin*w1 | cos*w2]),
                # then y[j] = prods[j,0:32] + prods[j,32:64].
                tb = tqtk[:, tbl_i, None, :, :].to_broadcast((P, 1, 2, HD))
                prods = qk_pool.tile([P, H, 2, HD], BF, tag="prods")
                nc.vector.tensor_tensor(
                    prods[:], qs[:, :, None, :].to_broadcast((P, H, 2, HD)),
                    tb.to_broadcast((P, H, 2, HD)), OP.mult,
                )
                nc.vector.tensor_tensor(
                    aug[:, :, 0:64].rearrange("p h (j d) -> p h j d", j=2),
                    prods[:, :, :, 0:32], prods[:, :, :, 32:64], OP.add,
                )
                aug_fn(aug)
                return nc.sync.dma_start(
                    dst_dram[t8 * P:(t8 + 1) * P, :],
                    aug[:].rearrange("p h d -> p (h d)"),
                )

            mark('qkv_loop')
            qk_process.flip = 0

            k_store = {}

            def do_k(t8):
                k_src = proj_corrected(wk_sb, t8, 1,
                                       bias_off=D if with_b1 else None)

                def k_aug(aug, t8=t8):
                    nc.vector.tensor_copy(
                        out=aug[:, :, 64:67],
                        in_=coords_sb[:, t8:t8 + 1, :].to_broadcast((P, H, 3)),
                    )

                k_store[t8] = qk_process(k_src, 4 + t8, t8, k_aug, kaug_d)

            def do_q(t4):
                q_src = proj_corrected(wq_sb, t4, 0,
                                       bias_off=0 if with_b1 else None)

                def q_aug(aug):
                    nc.vector.tensor_copy(
                        out=aug[:, :, 64:67],
                        in_=cq_sb[:, t4, :].rearrange("p (h c) -> p h c", h=H),
                    )

                qk_process(q_src, t4, t4, q_aug, qaug_d)

            def do_v(t8):
                proj_corrected(wv_sb, t8, 2,
                               bias_off=2 * D if with_b1 else None,
                               out_ap=vtil[:, t8, :, 0:64])

            # k and q early (attention transposes depend on them), v late:
            # its short evac chain lets attention score matmuls interleave
            # with the v-projection tail on PE.
            do_k(0)
            do_k(1)
            for t8 in range(4):
                do_q(t8)
                do_k(t8 + 2)
            do_k(6)
            do_v(0)
            do_k(7)
            do_v(1)
            for t8 in range(2, 8):
                do_v(t8)

        mark('phaseD')
        # ==================== phase D: attention per head ===================
        ow_ctx = ExitStack()
        ow_pool = ow_ctx.enter_context(tc.tile_pool(name="ow", bufs=1))
        owT_sb = ow_pool.tile([P, 8, D], BF)
        ow_r = owT_d.rearrange("(o p) f -> p o f", p=P)
        ow_inst = nc.sync.dma_start(owT_sb[:], ow_r)
        # pin the 2MB prefetch into mid-phase-C DMA slack so its transfer
        # cannot starve the latency-critical head transposes at the boundary
        tile.add_dep_helper(ow_inst.ins, k_store[2].ins,
                            reason="owT prefetch after mid-C")
        with tc.tile_pool(name="att", bufs=2) as att_pool, \
             tc.tile_pool(name="apsum", bufs=3, space="PSUM") as apsum, \
             tc.tile_pool(name="opsum", bufs=2, space="PSUM") as opsum:
            kT2 = qT2 = None
            xr_r = x_d[0:MY].rearrange("(o p) f -> p o f", p=P)
            for h in range(H):
                if h in (9, 11, 13, 15):
                    # residual fp32 rows (needed at phase E); spread between
                    # late heads so they never block the head transposes
                    i = (h - 9) // 2
                    nc.scalar.dma_start(x_res[:, i:i + 1], xr_r[:, i:i + 1])

                if h % 2 == 0:
                    kT2 = att_pool.tile([P, 2, N], BF, tag="kT")
                    nc.sync.dma_start_transpose(
                        kT2[:], kaug_d[:, h * P:(h + 2) * P])
                    qT2 = att_pool.tile([P, 2, MY], BF, tag="qT")
                    nc.sync.dma_start_transpose(
                        qT2[:], qaug_d[:, h * P:(h + 2) * P])
                kT = kT2[:, h % 2]
                qT = qT2[:, h % 2]
                expT = att_pool.tile([P, 8, MY], BF, tag="expT")
                for kc2 in range(4):
                    s_ps = apsum.tile([P, 2, MY], F32, tag="s_ps")
                    for j in range(2):
                        nc.tensor.matmul(
                            s_ps[:, j],
                            kT[:, (2 * kc2 + j) * P:(2 * kc2 + j + 1) * P], qT,
                            start=True, stop=True,
                        )
                    nc.scalar.activation(
                        expT[:, 2 * kc2:2 * kc2 + 2, :], s_ps[:],
                        AF.Exp, scale=0.125
                    )
                o_ps = opsum.tile([65, MY], F32, tag="o_ps")
                for kc in range(8):
                    nc.tensor.matmul(
                        o_ps[:], vtil[:, kc, h, :], expT[:, kc, :],
                        start=(kc == 0), stop=(kc == 7),
                    )
                rec = att_pool.tile([1, MY], F32, tag="rec")
                nc.vector.reciprocal(rec[:], o_ps[64:65, :])
                bc = att_pool.tile([64, MY], F32, tag="bc")
                nc.gpsimd.partition_broadcast(bc[:], rec[:])
                nc.vector.tensor_tensor(
                    oT_all[(h % 2) * 64:(h % 2) * 64 + 64, h // 2, :],
                    o_ps[0:64, :], bc[:], OP.mult,
                )

        mark('phaseE')
        # ================== phase E: out-proj + residual ====================
        with tc.tile_pool(name="ebuf", bufs=2) as ebuf, \
             tc.tile_pool(name="epsum", bufs=4, space="PSUM") as epsum:
            for qc in range(4):
                for eh in range(2):
                    xp = epsum.tile([P, 512], F32, tag="xp")
                    for jc in range(8):
                        nc.tensor.matmul(
                            xp[:],
                            oT_all[:, jc, qc * P:(qc + 1) * P],
                            owT_sb[:, jc, eh * 512:(eh + 1) * 512],
                            start=(jc == 0), stop=(jc == 7),
                        )
                    sl = slice(eh * 512, (eh + 1) * 512)
                    if with_bo1:
                        xb = ebuf.tile([P, 512], F32, tag="xpb")
                        nc.vector.tensor_tensor(xb[:], xp[:], bo1r_sb[:, sl],
                                                OP.add)
                        nc.vector.tensor_tensor(
                            x1_sb[:, qc, sl], x_res[:, qc, sl], xb[:], OP.add
                        )
                    else:
                        nc.vector.tensor_tensor(
                            x1_sb[:, qc, sl], x_res[:, qc, sl], xp[:], OP.add
                        )

        ow_ctx.close()
        mark('phaseF')
        # =================== phases F+G: LN2 + MLP up =======================
        with tc.tile_pool(name="mlp_pers", bufs=1) as mlp_pers:
            xh2T = mlp_pers.tile([P, 8, MY], BF)
            actT = mlp_pers.tile([P, 32, MY], BF)
            for qc in range(4):
                xh2_t = ln_pool.tile([P, D], BF, tag="xh2")
                layernorm_chunk(x1_sb[:, qc, :], xh2_t[:])
                nc.sync.dma_start(xh2_d[qc * P:(qc + 1) * P, :], xh2_t[:])
                nc.sync.dma_start_transpose(
                    xh2T[:, :, qc * P:(qc + 1) * P],
                    xh2_d[qc * P:(qc + 1) * P, :],
                )

            with tc.tile_pool(name="w2", bufs=3) as w2_pool, \
                 tc.tile_pool(name="sil", bufs=2) as sil_pool, \
                 tc.tile_pool(name="gpsum", bufs=4, space="PSUM") as gpsum:
                for jj in range(32):
                    w2_t = w2_pool.tile([P, 8, 256], BF, tag="w2t")
                    nc.sync.dma_start(
                        w2_t[:],
                        w2T_d[:, jj, :].rearrange("(o p) f -> p o f", p=P),
                    )
                    ups = []
                    for half in range(2):
                        up = gpsum.tile([P, MY], F32, tag="u_ps")
                        if jj < 4:
                            # token-split groups: the first half only needs
                            # token chunks 0-1 of xh2T, so these matmuls can
                            # start while LN2 of chunks 2-3 is still going
                            for th in range(2):
                                tsl = slice(th * 256, (th + 1) * 256)
                                for dc in range(8):
                                    nc.tensor.matmul(
                                        up[:, tsl],
                                        w2_t[:, dc, half * P:(half + 1) * P],
                                        xh2T[:, dc, tsl],
                                        start=(dc == 0), stop=(dc == 7),
                                    )
                        else:
                            for dc in range(8):
                                nc.tensor.matmul(
                                    up[:],
                                    w2_t[:, dc, half * P:(half + 1) * P],
                                    xh2T[:, dc, :],
                                    start=(dc == 0), stop=(dc == 7),
                                )
                        ups.append(up)
                    sil = sil_pool.tile([P, MY], F32, tag="sil")
                    nc.scalar.activation(
                        sil[:], ups[0][:], AF.Silu, bias=b2_sb[:, jj:jj + 1]
                    )
                    nc.vector.scalar_tensor_tensor(
                        actT[:, jj, :], ups[1][:], b2_sb[:, jj + 32:jj + 33],
                        sil[:], op0=OP.add, op1=OP.mult,
                    )

            mark('phaseH')
            # ============ phase H: MLP down + residual + out ================
            with tc.tile_pool(name="wo", bufs=3) as wo_pool, \
                 tc.tile_pool(name="outp", bufs=3) as out_pool, \
                 tc.tile_pool(name="mpsum", bufs=1, space="PSUM") as mpsum:
                for eh in range(2):
                    sl = slice(eh * 512, (eh + 1) * 512)
                    mps = [mpsum.tile([P, 512], F32, tag=f"m_ps{qc}",
                                      name=f"m_ps{qc}_{eh}")
                           for qc in range(4)]
                    for h4 in range(8):
                        wo_t = wo_pool.tile([P, 4, 512], BF, tag="wot")
                        nc.sync.dma_start(
                            wo_t[:],
                            woT_d[h4 * 512:(h4 + 1) * 512, sl].rearrange(
                                "(o p) f -> p o f", p=P),
                        )
                        for hi in range(4):
                            hc = h4 * 4 + hi
                            for qc in range(4):
                                nc.tensor.matmul(
                                    mps[qc][:],
                                    actT[:, hc, qc * P:(qc + 1) * P],
                                    wo_t[:, hi, :],
                                    start=(hc == 0), stop=(hc == 31),
                                )
                    for qc in range(4):
                        o_t = out_pool.tile([P, 512], F32, tag="outt")
                        if with_bo2:
                            ob = out_pool.tile([P, 512], F32, tag="outb")
                            nc.vector.tensor_tensor(ob[:], mps[qc][:],
                                                    bo2r_sb[:, sl], OP.add)
                            nc.vector.tensor_tensor(o_t[:], x1_sb[:, qc, sl],
                                                    ob[:], OP.add)
                        else:
                            nc.vector.tensor_tensor(
                                o_t[:], x1_sb[:, qc, sl], mps[qc][:], OP.add
                            )
                        nc.scalar.dma_start(out_d[qc * P:(qc + 1) * P, sl], o_t[:])

    mark('end')
    nc.compile()
    return nc


# ---------------------------------------------------------------------------
# host side
# ---------------------------------------------------------------------------

_prog_cache = {}


def _get_program(flags):
    if flags not in _prog_cache:
        _prog_cache[flags] = build_program(*flags)
    return _prog_cache[flags]


def kernel(**inputs):
    x = _f32(inputs["x"])
    coords = _f32(inputs["coords"])
    rope_pos = np.asarray(inputs["rope_pos"])
    ln1_w, ln1_b = _f32(inputs["ln1_w"]), _f32(inputs["ln1_b"])
    qkv_w, qkv_b = _f32(inputs["qkv_w"]), _f32(inputs["qkv_b"])
    qnw, knw = _f32(inputs["q_norm_w"]), _f32(inputs["k_norm_w"])
    sq_w, sk_w = _f32(inputs["sq_w"]), _f32(inputs["sk_w"])
    out_w, out_b = _f32(inputs["out_w"]), _f32(inputs["out_b"])
    ls1 = _f32(inputs["ls1_g"])
    ln2_w, ln2_b = _f32(inputs["ln2_w"]), _f32(inputs["ln2_b"])
    w12_w, w12_b = _f32(inputs["w12_w"]), _f32(inputs["w12_b"])
    wo_w, wo_b = _f32(inputs["wo_w"]), _f32(inputs["wo_b"])
    ls2 = _f32(inputs["ls2_g"])

    # ---- weight folding ----
    W1 = qkv_w * ln1_w[None, :]
    b1 = qkv_w @ ln1_b + qkv_b
    perm = np.empty(HD, np.int64)
    perm[:32] = np.arange(32) * 2
    perm[32:] = np.arange(32) * 2 + 1
    permD = np.concatenate([h * HD + perm for h in range(H)])
    Wq = W1[:D][permD]
    Wk = W1[D:2 * D][permD]
    Wv = W1[2 * D:]
    b1p = np.concatenate([b1[:D][permD], b1[D:2 * D][permD], b1[2 * D:]])
    qnw_p, knw_p = qnw[perm], knw[perm]

    half = 32
    inv_freq = 1.0 / THETA ** (np.arange(half, dtype=np.float32) / half)
    freqs = rope_pos.astype(np.float32)[:, None] * inv_freq
    cos, sin = np.cos(freqs), np.sin(freqs)
    def rope_tbl(w):
        t = np.empty((N, 2, 64), np.float32)
        t[:, 0, :32] = cos * w[None, :32]
        t[:, 0, 32:] = -sin * w[None, 32:]
        t[:, 1, :32] = sin * w[None, :32]
        t[:, 1, 32:] = cos * w[None, 32:]
        return t

    tq = rope_tbl(qnw_p)
    tk = rope_tbl(knw_p)

    A_cat = np.concatenate(
        [SP_SCALE * sq_w[h * HD:(h + 1) * HD].T @ sk_w[h * HD:(h + 1) * HD]
         for h in range(H)], 1)  # (3, 48)

    Wo1 = out_w * ls1[:, None]
    bo1 = ls1 * out_b
    W2 = w12_w * ln2_w[None, :]
    b2 = w12_w @ ln2_b + w12_b
    Wo2 = wo_w * ls2[:, None]
    bo2 = ls2 * wo_b

    with_b1 = bool(np.any(b1p != 0))
    with_bo1 = bool(np.any(bo1 != 0))
    with_bo2 = bool(np.any(bo2 != 0))
    flags = (with_b1, with_bo1, with_bo2)
    nc = _get_program(flags)

    Wqb = _bf(Wq).astype(np.float32)
    Wkb = _bf(Wk).astype(np.float32)
    Wvb = _bf(Wv).astype(np.float32)
    csum = np.concatenate([Wqb.sum(1), Wkb.sum(1), Wvb.sum(1)])  # (3D,)
    w2T = _bf(W2.T)  # (D, 2*HID)
    w2pair = np.empty((D, 32, 256), bf16)
    w2pair[:, :, :128] = w2T[:, :HID].reshape(D, 32, 128)
    w2pair[:, :, 128:] = w2T[:, HID:].reshape(D, 32, 128)
    shared = {
        "a_cat": _bf(A_cat),
        "wqT": _bf(Wq.T), "wkT": _bf(Wk.T), "wvT": _bf(Wv.T),
        "owT": _bf(Wo1.T),
        "w2T": np.ascontiguousarray(w2pair),
        "csum_rep": _bf(np.broadcast_to(csum[None, :], (P, 3 * D))),
        "b2": _f32(b2.reshape(2 * HID // P, P).T),
        "woT": _bf(Wo2.T),
    }
    if with_b1:
        shared["b1rep"] = _f32(np.broadcast_to(b1p[None, :], (P, 3 * D)))
    if with_bo1:
        shared["bo1rep"] = _f32(np.broadcast_to(bo1[None, :], (P, D)))
    if with_bo2:
        shared["bo2rep"] = _f32(np.broadcast_to(bo2[None, :], (P, D)))

    in_maps = []
    for c in range(NCORES):
        b, r = c // 2, c % 2
        rows = np.concatenate([np.arange(r * MY, (r + 1) * MY),
                               np.arange(0, r * MY),
                               np.arange((r + 1) * MY, N)])
        m = dict(shared)
        m["x"] = _f32(x[b][rows])
        m["x_bf"] = _bf(x[b][rows])
        m["coords_tm"] = _bf(coords[b][rows])
        m["coordsT"] = _bf(coords[b][rows].T)
        m["tq"] = _bf(tq[r * MY:(r + 1) * MY])
        m["tk"] = _bf(tk[rows])
        in_maps.append(m)

    res = run_bass_kernel_spmd(nc, in_maps, core_ids=list(range(NCORES)),
                               trace=bool(int(os.environ.get("K_TRACE", "0"))))
    out = np.empty((B, N, D), np.float32)
    for c in range(NCORES):
        b, r = c // 2, c % 2
        out[b, r * MY:(r + 1) * MY] = res.results[c]["out"]
    kernel.last_result = res
    return out

